# revision 1
# baseline (speedup 1.0000x reference)
"""Trainium2 Bass kernel for a 2-layer GCN encoder + edge dot-product decoder.

Math (matches the PyG-style reference):
    deg  = in-degree(dst)+1 (self loops), dinv = rsqrt(deg)
    A~[d,s] = dinv[s]*dinv[d] over edges+self-loops
    H1 = (A~ @ X) @ W1 + b1          (aggregate-first ordering)
    Z  = (A~ @ relu(H1) @ W2) + b2
    logits[e] = <Z[src_e], Z[dst_e]>

Distribution over 8 NeuronCores:
  - nodes are sharded contiguously (6250/core, padded to 6400); edges are
    partitioned by destination-node owner; X is replicated so the layer-1
    gather is local; the layer-2/decoder operand tables (h2', z) are
    AllGathered (12.8 MB total each) between phases.
  - the segment-sum scatter is computed on the Tensor Engine: for each
    128-edge block a [128e x 128slot] matrix S with S[e, slot(dst_e)] =
    norm_e is built on the Vector Engine (iota + is_equal*mult), and
    PSUM accumulates S.T @ Xg over the blocks of each 128-dst tile.
"""

import os

# The Bass runner needs the axon jax platform; a harness that pins
# JAX_PLATFORMS=cpu for its own reference would break it.
if os.environ.get("JAX_PLATFORMS") == "cpu":
    os.environ.pop("JAX_PLATFORMS")

import numpy as np

from concourse import bass, bacc, mybir, bass_utils
import concourse.tile as tile

# ---------------------------------------------------------------- sizes
N_NODES = 50000
N_EDGES = 400000
D_IN, D_H, D_OUT = 600, 628, 64
C = 8            # cores
P = 128          # partitions / tile width

BALANCE = True
XG_BUFS = 12
MG_BUFS = 12
DEC_BUFS = 16

F16 = mybir.dt.float16
F32 = mybir.dt.float32
I32 = mybir.dt.int32


def _chunks(total, step=128):
    out = []
    o = 0
    while o < total:
        w = min(step, total - o)
        out.append((o, w))
        o += w
    return out


def _cfg():
    npc = N_NODES // C                      # real nodes per core
    tiles = -(-npc // P)                    # dst tiles per core
    npad = tiles * P                        # padded nodes per core
    epc = N_EDGES // C                      # decoder edges per core
    nbd = -(-epc // P)                      # decoder blocks per core
    return dict(
        npc=npc, tiles=tiles, npad=npad, epc=epc, nbd=nbd,
        kch=_chunks(D_IN), mch=_chunks(D_H),
        groups=[list(range(i, min(i + 4, tiles))) for i in range(0, tiles, 4)],
    )


# ---------------------------------------------------------------- host preprocessing
def _assign_nodes(d_all, N, tiles):
    """Balance aggregation work: LPT-assign nodes to C*tiles buckets of <=128,
    minimizing the max per-bucket edge count. Returns per-node (core, tile, slot)."""
    import heapq
    w = np.bincount(d_all, minlength=N)
    nb = C * tiles
    heap = [(0, b) for b in range(nb)]
    heapq.heapify(heap)
    cnt = np.zeros(nb, np.int64)
    nodec = np.empty(N, np.int64)
    nodet = np.empty(N, np.int64)
    nodesl = np.empty(N, np.int64)
    for n in np.argsort(-w, kind="stable"):
        while True:
            wt, b = heapq.heappop(heap)
            if cnt[b] < P:
                break
        nodec[n] = b // tiles
        nodet[n] = b % tiles
        nodesl[n] = cnt[b]
        cnt[b] += 1
        if cnt[b] < P:
            heapq.heappush(heap, (wt + int(w[n]), b))
    return nodec, nodet, nodesl


def _preprocess(x, edge_index, W1, b1, W2, b2):
    cfg = _cfg()
    npc, tiles, npad, epc, nbd = (
        cfg["npc"], cfg["tiles"], cfg["npad"], cfg["epc"], cfg["nbd"])
    N = x.shape[0]
    src = edge_index[0].astype(np.int64)
    dst = edge_index[1].astype(np.int64)
    loop = np.arange(N, dtype=np.int64)
    s_all = np.concatenate([src, loop])
    d_all = np.concatenate([dst, loop])
    deg = np.bincount(d_all, minlength=N).astype(np.float64)
    dinv = 1.0 / np.sqrt(deg)
    norm = (dinv[s_all] * dinv[d_all]).astype(np.float32)

    if BALANCE:
        nodec, nodet, nodesl = _assign_nodes(d_all, N, tiles)
    else:
        nodec = np.arange(N) // npc
        nodet = (np.arange(N) % npc) // P
        nodesl = (np.arange(N) % npc) % P
    staged = nodec * npad + nodet * P + nodesl
    core = nodec[d_all]
    tl = nodet[d_all]
    slot = nodesl[d_all]
    gt = core * tiles + tl
    order = np.argsort(gt, kind="stable")
    counts = np.bincount(gt, minlength=C * tiles).reshape(C, tiles)
    B = np.maximum((-(-counts // P)).max(axis=0), 1).astype(np.int64)  # blocks per tile
    off = np.zeros(tiles + 1, dtype=np.int64)
    off[1:] = np.cumsum(B)
    SB = int(off[-1])

    start_gt = np.zeros(C * tiles + 1, dtype=np.int64)
    start_gt[1:] = np.cumsum(counts.reshape(-1))
    rank = np.arange(len(order)) - start_gt[gt[order]]
    col = off[tl[order]] + rank // P
    pp = rank % P
    cs = core[order]
    srcs = s_all[order]

    meta = np.zeros((C, P, 2 * SB), dtype=np.float32)
    src1 = np.zeros((C, P, SB), dtype=np.int32)
    src2 = np.zeros((C, P, SB), dtype=np.int32)
    meta[cs, pp, 2 * col] = slot[order].astype(np.float32)
    meta[cs, pp, 2 * col + 1] = norm[order].astype(np.float32)
    src1[cs, pp, col] = srcs.astype(np.int32)
    src2[cs, pp, col] = staged[srcs].astype(np.int32)

    # decoder: original edge order, contiguous slices per core
    e = np.arange(N_EDGES)
    cd = e // epc
    wi = e % epc
    bd = wi // P
    pd = wi % P
    didx = np.zeros((C, P, 2 * nbd), dtype=np.int32)
    didx[cd, pd, 2 * bd] = staged[src].astype(np.int32)
    didx[cd, pd, 2 * bd + 1] = staged[dst].astype(np.int32)

    iota = np.broadcast_to(np.arange(P, dtype=np.float16), (P, P)).copy()
    ident = np.eye(P, dtype=np.float16)

    shared = {
        "xt": np.ascontiguousarray(x.astype(np.float16)),
        "w1": np.ascontiguousarray(W1.astype(np.float16)),
        "w2": np.ascontiguousarray(W2.astype(np.float16)),
        "b1c": np.ascontiguousarray(b1.astype(np.float32).reshape(D_H, 1)),
        "b2r": np.ascontiguousarray(
            np.broadcast_to(b2.astype(np.float32), (P, D_OUT))),
        "iota": iota,
        "ident": ident,
    }
    in_maps = []
    for c in range(C):
        m = dict(shared)
        m["meta"] = np.ascontiguousarray(meta[c])
        m["src1"] = np.ascontiguousarray(src1[c])
        m["src2"] = np.ascontiguousarray(src2[c])
        m["didx"] = np.ascontiguousarray(didx[c])
        in_maps.append(m)
    return in_maps, [int(b) for b in B], SB, cfg


# ---------------------------------------------------------------- device program
def _build(B, SB, cfg, ph=9, skip_gather=False):
    npc, tiles, npad, epc, nbd = (
        cfg["npc"], cfg["tiles"], cfg["npad"], cfg["epc"], cfg["nbd"])
    kch, mch, groups = cfg["kch"], cfg["mch"], cfg["groups"]
    off = np.zeros(tiles + 1, dtype=np.int64)
    off[1:] = np.cumsum(B)

    nc = bacc.Bacc("TRN2", target_bir_lowering=False, debug=False,
                   enable_asserts=False, num_devices=C)

    xt = nc.dram_tensor("xt", [N_NODES, D_IN], F16, kind="ExternalInput")
    w1 = nc.dram_tensor("w1", [D_IN, D_H], F16, kind="ExternalInput")
    w2 = nc.dram_tensor("w2", [D_H, D_OUT], F16, kind="ExternalInput")
    b1c = nc.dram_tensor("b1c", [D_H, 1], F32, kind="ExternalInput")
    b2r = nc.dram_tensor("b2r", [P, D_OUT], F32, kind="ExternalInput")
    iota_d = nc.dram_tensor("iota", [P, P], F16, kind="ExternalInput")
    ident_d = nc.dram_tensor("ident", [P, P], F16, kind="ExternalInput")
    meta_d = nc.dram_tensor("meta", [P, 2 * SB], F32, kind="ExternalInput")
    src1_d = nc.dram_tensor("src1", [P, SB], I32, kind="ExternalInput")
    src2_d = nc.dram_tensor("src2", [P, SB], I32, kind="ExternalInput")
    didx_d = nc.dram_tensor("didx", [P, 2 * nbd], I32, kind="ExternalInput")
    logits_d = nc.dram_tensor("logits", [P, nbd], F32, kind="ExternalOutput")

    rg = [list(range(C))]

    with tile.TileContext(nc) as tc:
        with (
            tc.tile_pool(name="const", bufs=1) as constp,
            tc.tile_pool(name="meta", bufs=1) as metap,
            tc.tile_pool(name="xg", bufs=XG_BUFS) as xgp,
            tc.tile_pool(name="sblk", bufs=4) as sp,
            tc.tile_pool(name="xagg", bufs=2) as xaggp,
            tc.tile_pool(name="kxn", bufs=2) as kxnp,
            tc.tile_pool(name="h1r", bufs=2) as h1rp,
            tc.tile_pool(name="h2s", bufs=2) as h2sp,
            tc.tile_pool(name="mg", bufs=MG_BUFS) as mgp,
            tc.tile_pool(name="zz", bufs=3) as zp,
            tc.tile_pool(name="dec", bufs=DEC_BUFS) as decp,
            tc.tile_pool(name="pacc", bufs=2, space="PSUM") as pacc,
            tc.tile_pool(name="ptp", bufs=2, space="PSUM") as ptp,
            tc.tile_pool(name="ph", bufs=2, space="PSUM") as php,
            tc.tile_pool(name="dram", bufs=1, space="DRAM") as dramp,
        ):
            # ---- constants / persistent tables
            w1sb = []
            for k, (k0, kw) in enumerate(kch):
                t = constp.tile([kw, D_H], F16, name=f"w1sb{k}", tag=f"w1sb{k}")
                nc.sync.dma_start(out=t[:], in_=w1[k0:k0 + kw, :])
                w1sb.append(t)
            w2sb = []
            b1sb = []
            for m, (m0, mw) in enumerate(mch):
                t = constp.tile([mw, D_OUT], F16, name=f"w2sb{m}", tag=f"w2sb{m}")
                nc.sync.dma_start(out=t[:], in_=w2[m0:m0 + mw, :])
                w2sb.append(t)
                bt = constp.tile([mw, 1], F32, name=f"b1sb{m}", tag=f"b1sb{m}")
                nc.sync.dma_start(out=bt[:], in_=b1c[m0:m0 + mw, :])
                b1sb.append(bt)
            b2sb = constp.tile([P, D_OUT], F32, name="b2sb", tag="b2sb")
            nc.sync.dma_start(out=b2sb[:], in_=b2r[:, :])
            iot = constp.tile([P, P], F16, name="iot", tag="iot")
            nc.sync.dma_start(out=iot[:], in_=iota_d[:, :])
            idn = constp.tile([P, P], F16, name="idn", tag="idn")
            nc.sync.dma_start(out=idn[:], in_=ident_d[:, :])
            meta_all = metap.tile([P, 2 * SB], F32, name="meta_all", tag="meta_all")
            nc.sync.dma_start(out=meta_all[:], in_=meta_d[:, :])
            src1_all = metap.tile([P, SB], I32, name="src1_all", tag="src1_all")
            nc.sync.dma_start(out=src1_all[:], in_=src1_d[:, :])
            src2_all = metap.tile([P, SB], I32, name="src2_all", tag="src2_all")
            nc.sync.dma_start(out=src2_all[:], in_=src2_d[:, :])
            didx_all = metap.tile([P, 2 * nbd], I32, name="didx_all", tag="didx_all")
            nc.sync.dma_start(out=didx_all[:], in_=didx_d[:, :])

            h2loc = dramp.tile([npad, D_OUT], F16, name="h2loc", tag="h2loc")
            h2full = dramp.tile([C * npad, D_OUT], F16, name="h2full",
                                tag="h2full", addr_space="Shared")
            zloc = dramp.tile([npad, D_OUT], F16, name="zloc", tag="zloc")
            zfull = dramp.tile([C * npad, D_OUT], F16, name="zfull",
                               tag="zfull", addr_space="Shared")

            def build_s(o):
                s_t = sp.tile([P, P], F16, name="s_t", tag="s_t")
                nc.vector.tensor_scalar(
                    out=s_t[:], in0=iot[:],
                    scalar1=meta_all[:, 2 * o:2 * o + 1],
                    scalar2=meta_all[:, 2 * o + 1:2 * o + 2],
                    op0=mybir.AluOpType.is_equal,
                    op1=mybir.AluOpType.mult)
                return s_t

            # ---- layer 1: scatter into Xagg tiles, transpose, GEMMs
            for g, tlist in enumerate(groups):
                gw = len(tlist) * P
                g0 = tlist[0] * P      # local node offset of group
                kxn = [kxnp.tile([P, gw], F16, name=f"kxn{k}", tag=f"kxn{k}")
                       for k in range(len(kch))]
                for j, t in enumerate(tlist):
                    acc = pacc.tile([P, D_IN], F32, name="acc", tag="acc")
                    for b in range(B[t]):
                        o = int(off[t]) + b
                        xgt = xgp.tile([P, D_IN], F16, name="xgt", tag="xgt")
                        if skip_gather:
                            nc.gpsimd.memset(xgt[:], 0.0)
                        else:
                            nc.gpsimd.indirect_dma_start(
                                out=xgt[:], out_offset=None, in_=xt[:],
                                in_offset=bass.IndirectOffsetOnAxis(
                                    ap=src1_all[:, o:o + 1], axis=0))
                        s_t = build_s(o)
                        nc.tensor.matmul(acc[:, 0:512], lhsT=s_t[:], rhs=xgt[:, 0:512],
                                         start=(b == 0), stop=(b == B[t] - 1))
                        nc.tensor.matmul(acc[:, 512:D_IN], lhsT=s_t[:],
                                         rhs=xgt[:, 512:D_IN],
                                         start=(b == 0), stop=(b == B[t] - 1))
                    xaggsb = xaggp.tile([P, D_IN], F16, name="xaggsb", tag="xaggsb")
                    nc.scalar.copy(out=xaggsb[:], in_=acc[:])
                    for k, (k0, kw) in enumerate(kch):
                        tp = ptp.tile([P, P], F16, name="tp", tag="tp")
                        nc.tensor.transpose(out=tp[:kw, :], in_=xaggsb[:, k0:k0 + kw],
                                            identity=idn[:])
                        nc.vector.tensor_copy(
                            out=kxn[k][:kw, j * P:(j + 1) * P], in_=tp[:kw, :])
                # GEMM1 + relu, feat-major
                h1r = [h1rp.tile([mw, gw], F16, name=f"h1r{m}", tag=f"h1r{m}")
                       for m, (m0, mw) in enumerate(mch)]
                for m, (m0, mw) in enumerate(mch):
                    hp = php.tile([P, gw], F32, name="hp", tag="hp")
                    for k, (k0, kw) in enumerate(kch):
                        nc.tensor.matmul(hp[:mw, :], lhsT=w1sb[k][:, m0:m0 + mw],
                                         rhs=kxn[k][:kw, :],
                                         start=(k == 0), stop=(k == len(kch) - 1))
                    nc.scalar.activation(out=h1r[m][:], in_=hp[:mw, :],
                                         func=mybir.ActivationFunctionType.Relu,
                                         bias=b1sb[m][:], scale=1.0)
                # GEMM2, feat-major [64, gw]
                h2p = php.tile([P, gw], F32, name="h2p", tag="hp")
                for m, (m0, mw) in enumerate(mch):
                    nc.tensor.matmul(h2p[:D_OUT, :], lhsT=w2sb[m][:], rhs=h1r[m][:],
                                     start=(m == 0), stop=(m == len(mch) - 1))
                h2sb = h2sp.tile([D_OUT, gw], F16, name="h2sb", tag="h2sb")
                nc.scalar.copy(out=h2sb[:], in_=h2p[:D_OUT, :])
                for j in range(len(tlist)):
                    tp2 = ptp.tile([P, P], F16, name="tp2", tag="tp")
                    nc.tensor.transpose(out=tp2[:, :D_OUT],
                                        in_=h2sb[:, j * P:(j + 1) * P],
                                        identity=idn[:D_OUT, :D_OUT])
                    h2row = zp.tile([P, D_OUT], F16, name="h2row", tag="h2row")
                    nc.vector.tensor_copy(out=h2row[:], in_=tp2[:, :D_OUT])
                    r0 = g0 + j * P
                    nc.sync.dma_start(out=h2loc[r0:r0 + P, :], in_=h2row[:])

            if ph >= 2:
                nc.gpsimd.collective_compute(
                    "AllGather", mybir.AluOpType.bypass, replica_groups=rg,
                    ins=[h2loc[:].opt()], outs=[h2full[:].opt()])

            # ---- layer 2: scatter h2' into z (node-major), + b2
            for t in range(tiles if ph >= 3 else 0):
                acc2 = ptp.tile([P, D_OUT], F32, name="acc2", tag="tp")
                for b in range(B[t]):
                    o = int(off[t]) + b
                    mg = mgp.tile([P, D_OUT], F16, name="mg", tag="mg")
                    nc.gpsimd.indirect_dma_start(
                        out=mg[:], out_offset=None, in_=h2full[:],
                        in_offset=bass.IndirectOffsetOnAxis(
                            ap=src2_all[:, o:o + 1], axis=0))
                    s_t = build_s(o)
                    nc.tensor.matmul(acc2[:], lhsT=s_t[:], rhs=mg[:],
                                     start=(b == 0), stop=(b == B[t] - 1))
                zsb = zp.tile([P, D_OUT], F16, name="zsb", tag="zsb")
                nc.vector.tensor_add(out=zsb[:], in0=acc2[:], in1=b2sb[:])
                nc.sync.dma_start(out=zloc[t * P:(t + 1) * P, :], in_=zsb[:])

            if ph >= 4:
                nc.gpsimd.collective_compute(
                    "AllGather", mybir.AluOpType.bypass, replica_groups=rg,
                    ins=[zloc[:].opt()], outs=[zfull[:].opt()])

            # ---- decoder
            lacc = decp.tile([P, nbd], F32, name="lacc", tag="lacc", bufs=1)
            if ph < 5:
                nc.gpsimd.memset(lacc[:], 0.0)
            for b in range(nbd if ph >= 5 else 0):
                zs = decp.tile([P, D_OUT], F16, name="zs", tag="zs")
                zd = decp.tile([P, D_OUT], F16, name="zd", tag="zd")
                nc.gpsimd.indirect_dma_start(
                    out=zs[:], out_offset=None, in_=zfull[:],
                    in_offset=bass.IndirectOffsetOnAxis(
                        ap=didx_all[:, 2 * b:2 * b + 1], axis=0))
                nc.gpsimd.indirect_dma_start(
                    out=zd[:], out_offset=None, in_=zfull[:],
                    in_offset=bass.IndirectOffsetOnAxis(
                        ap=didx_all[:, 2 * b + 1:2 * b + 2], axis=0))
                prod = decp.tile([P, D_OUT], F32, name="prod", tag="prod")
                nc.vector.tensor_mul(out=prod[:], in0=zs[:], in1=zd[:])
                nc.vector.reduce_sum(out=lacc[:, b:b + 1], in_=prod[:],
                                     axis=mybir.AxisListType.X)
            nc.sync.dma_start(out=logits_d[:, :], in_=lacc[:])

    nc.compile()
    return nc


# ---------------------------------------------------------------- entry point
_CACHE = {}


def kernel(x, edge_index, W1, b1, W2, b2):
    x = np.asarray(x)
    edge_index = np.asarray(edge_index)
    in_maps, B, SB, cfg = _preprocess(x, edge_index,
                                      np.asarray(W1), np.asarray(b1),
                                      np.asarray(W2), np.asarray(b2))
    key = (tuple(B), SB)
    if key not in _CACHE:
        _CACHE[key] = _build(B, SB, cfg)
    nc = _CACHE[key]
    res = bass_utils.run_bass_kernel_spmd(nc, in_maps, core_ids=list(range(C)))
    epc, nbd = cfg["epc"], cfg["nbd"]
    out = np.empty(N_EDGES, dtype=np.float32)
    for c in range(C):
        lg = res.results[c]["logits"]          # [P, nbd]
        out[c * epc:(c + 1) * epc] = lg.T.reshape(-1)[:epc]
    return out

